# revision 17
# baseline (speedup 1.0000x reference)
"""Category-routed 2-layer MLP (MoE-style routing) on 8 Trainium2 cores.

Problem: out[i] = relu(x[i] @ W1[c] + b1[c]) @ W2[c] + b2[c], c = cat_ids[i],
with B=131072 tokens, C=16 categories, D_IN=256, D_H=1024, D_OUT=256.

Sharding strategy (chosen; spec hint suggests token-data-parallel, but
category-sharding cuts per-core weight traffic 8x):
  - Each core owns 2 whole categories (16 cats / 8 cores). Tokens are routed
    (gathered) to their category's core on the host as part of sharding, and
    handed to the device already transposed ([D_IN, T] layout) so the
    contraction dim sits on SBUF partitions.
  - Per-core segments are padded to a uniform compile-time capacity so a
    single SPMD program serves all 8 cores. Padded columns are zero.
  - The device runs the fused two-layer MLP per 512-token chunk:
      L1: psum[h,t] += W1_kd_tile.T @ xT_tile   (fp32r, full PE rate)
      relu+bias on ScalarE (PSUM -> SBUF, fp32r, bias per-partition)
      L2: psum[t,o] += hT_tile.T @ W2_tile      (fp32r)
      bias2 add on VectorE (PSUM -> SBUF, broadcast bias tile)
      DMA out rows (already in [t, o] layout - no output transpose).
  - Host scatters the per-core packed outputs back to original token order.

Matmuls run in float32r (TF32-like reduced precision at full PE rate,
absmax error ~1e-4 - measured ~15x better than bf16).
"""

import numpy as np
from contextlib import ExitStack

import concourse.bacc as bacc
import concourse.tile as tile
from concourse import mybir
from concourse.bass_utils import run_bass_kernel_spmd

N_CORES = 8
P = 128          # SBUF partitions
CHUNK = 512      # tokens per inner tile (PSUM bank = 512 fp32)
F32 = mybir.dt.float32
F32R = mybir.dt.float32r
RELU = mybir.ActivationFunctionType.Relu


def build_program(seg_caps, d_in, d_h, d_out, repeat=1, relu_acts=8,
                  raw_f32r=True, ps1_bufs=4, ps2_bufs=4, chunk=CHUNK,
                  hp_bufs=2, deep_io=False):
    """Emit the SPMD program for one core.

    seg_caps: list of per-segment token capacities (each a multiple of CHUNK).
    Inputs (per core): xT [d_in, T] f32 (transposed, packed, zero-padded),
      W1 [S, d_in, d_h], b1 [S, d_h], W2 [S, d_h, d_out], b2bc [S, 128, d_out].
    Output: out [T, d_out] f32, token rows in packed order.
    """
    n_seg = len(seg_caps)
    T = sum(seg_caps)
    KD = d_in // P    # contraction tiles for layer 1
    MH = d_h // P     # h tiles (layer-1 out partitions / layer-2 contraction)
    NT = CHUNK // P   # token sub-tiles for layer 2

    nc = bacc.Bacc("TRN2", target_bir_lowering=False, debug=False,
                   num_devices=N_CORES)
    # When raw_f32r: declare matmul inputs as float32r in DRAM (same 4-byte
    # layout; PE applies its own operand rounding), skipping on-chip rounding
    # copies entirely.
    XDT = F32R if raw_f32r else F32
    xT = nc.declare_dram_parameter("xT", [d_in, T], XDT, isOutput=False)
    W1 = nc.declare_dram_parameter("W1", [n_seg, d_in, d_h], XDT, isOutput=False)
    b1 = nc.declare_dram_parameter("b1", [n_seg, d_h], F32, isOutput=False)
    W2 = nc.declare_dram_parameter("W2", [n_seg, d_h, d_out], XDT, isOutput=False)
    b2bc = nc.declare_dram_parameter("b2bc", [n_seg, P, d_out], F32, isOutput=False)
    out = nc.declare_dram_parameter("out", [T, d_out], F32, isOutput=True)

    # DRAM-side access-pattern views
    xT_v = xT.ap().rearrange("(kd p) t -> p kd t", p=P)            # [P, KD, T]
    w1_v = W1.ap().rearrange("s (kd p) h -> s p kd h", p=P)        # [S, P, KD, d_h]
    w2_v = W2.ap().rearrange("s (kh p) o -> s p kh o", p=P)        # [S, P, MH, d_out]
    b1_v = b1.ap().rearrange("s (mh p) -> s p mh", p=P)            # [S, P, MH]
    b2_v = b2bc.ap()                                               # [S, P, d_out]
    out_v = out.ap()

    # chunk list: as many `chunk`-sized tiles as fit, then a 512 tail
    chunk_list = []
    for s in range(n_seg):
        off_t, rem = sum(seg_caps[:s]), seg_caps[s]
        while rem > 0:
            sz = min(chunk, rem)
            chunk_list.append((s, off_t, sz))
            off_t += sz
            rem -= sz

    if chunk > 512:
        # [P, chunk] fp32 psum spans chunk/512 banks
        ps1_bufs = min(ps1_bufs, 6 * 512 // chunk)
        ps2_bufs = min(ps2_bufs, 2)
        xp_bufs, op_bufs = 3, 3
    else:
        xp_bufs, op_bufs = (6, 6) if deep_io else (4, 4)

    with tile.TileContext(nc) as tc, ExitStack() as ctx:
        const = ctx.enter_context(tc.tile_pool(name="const", bufs=1))
        xpool = ctx.enter_context(tc.tile_pool(name="xp", bufs=xp_bufs))
        hpool = ctx.enter_context(tc.tile_pool(name="hp", bufs=hp_bufs))
        opool = ctx.enter_context(tc.tile_pool(name="op", bufs=op_bufs))
        ps1 = ctx.enter_context(tc.tile_pool(name="ps1", bufs=ps1_bufs, space="PSUM"))
        ps2 = ctx.enter_context(tc.tile_pool(name="ps2", bufs=ps2_bufs, space="PSUM"))

        if not raw_f32r:
            wstage = ctx.enter_context(tc.tile_pool(name="wstage", bufs=2))

        # Preload weights as fp32r. raw_f32r: plain HWDGE load (DRAM already
        # declared f32r). Else: f32 load + DVE rounding copy (SWDGE cast-DMA
        # runs on the slow Q7 software path - avoid).
        w1_sb, w2_sb, b1_sb, b2_sb = [], [], [], []
        for s in range(n_seg):
            w1_t = const.tile([P, KD, d_h], F32R, tag=f"w1_{s}")
            w2_t = const.tile([P, MH, d_out], F32R, tag=f"w2_{s}")
            if raw_f32r:
                nc.sync.dma_start(out=w1_t[:], in_=w1_v[s])
                nc.sync.dma_start(out=w2_t[:], in_=w2_v[s])
            else:
                st1 = wstage.tile([P, KD, d_h], F32, tag="wst1")
                nc.sync.dma_start(out=st1[:], in_=w1_v[s])
                nc.vector.tensor_copy(w1_t[:], st1[:])
                st2 = wstage.tile([P, MH, d_out], F32, tag="wst2")
                nc.sync.dma_start(out=st2[:], in_=w2_v[s])
                nc.vector.tensor_copy(w2_t[:], st2[:])
            w1_sb.append(w1_t)
            w2_sb.append(w2_t)
            b1_t = const.tile([P, MH], F32, tag=f"b1_{s}")
            nc.sync.dma_start(out=b1_t[:], in_=b1_v[s])
            b1_sb.append(b1_t)
            b2_t = const.tile([P, d_out], F32, tag=f"b2_{s}")
            nc.sync.dma_start(out=b2_t[:], in_=b2_v[s])
            b2_sb.append(b2_t)

        def emit_l2(hT, s, tok0, sz):
            # layer 2: out[t, o] = hT.T @ W2 + b2
            nt = sz // P
            ot = opool.tile([P, nt, d_out], F32, tag="ot")
            for tt in range(nt):
                pt2 = ps2.tile([P, d_out], F32, tag="ps2")
                for kh in range(MH):
                    nc.tensor.matmul(
                        pt2[:],
                        lhsT=hT[:, kh, tt * P:(tt + 1) * P],
                        rhs=w2_sb[s][:, kh, :],
                        start=(kh == 0), stop=(kh == MH - 1))
                nc.vector.tensor_add(ot[:, tt, :], pt2[:], b2_sb[s][:])
            dst = out_v[tok0:tok0 + sz, :].rearrange(
                "(nt p) o -> p nt o", p=P)
            nc.sync.dma_start(out=dst, in_=ot[:])

        for _rep in range(repeat):
            pending = None  # (hT, s, tok0, sz) awaiting layer 2
            for (s, tok0, sz) in chunk_list:
                if raw_f32r:
                    xt = xpool.tile([P, KD, sz], F32R, tag="xt")
                    nc.sync.dma_start(
                        out=xt[:], in_=xT_v[:, :, tok0:tok0 + sz])
                else:
                    # load x chunk (HWDGE), round f32 -> f32r on DVE
                    xs = xpool.tile([P, KD, sz], F32, tag="xs")
                    nc.sync.dma_start(
                        out=xs[:], in_=xT_v[:, :, tok0:tok0 + sz])
                    xt = xpool.tile([P, KD, sz], F32R, tag="xt")
                    nc.vector.tensor_copy(xt[:], xs[:])

                # layer 1: hT[h, t] = relu(W1.T @ xT + b1)
                hT = hpool.tile([P, MH, sz], F32R, tag="hT")
                for mh in range(MH):
                    pt = ps1.tile([P, sz], F32, tag="ps1")
                    for half in range(sz // 512):
                        hs = slice(half * 512, (half + 1) * 512)
                        for kd in range(KD):
                            nc.tensor.matmul(
                                pt[:, hs],
                                lhsT=w1_sb[s][:, kd, mh * P:(mh + 1) * P],
                                rhs=xt[:, kd, hs],
                                start=(kd == 0), stop=(kd == KD - 1))
                    if mh < relu_acts:
                        nc.scalar.activation(
                            hT[:, mh, :], pt[:], RELU,
                            bias=b1_sb[s][:, mh:mh + 1])
                    else:
                        nc.vector.tensor_scalar(
                            hT[:, mh, :], pt[:],
                            b1_sb[s][:, mh:mh + 1], 0.0,
                            mybir.AluOpType.add, mybir.AluOpType.max)

                # software pipeline: layer 2 runs one chunk behind, so PE
                # never waits on this chunk's relu chain.
                if pending is not None:
                    emit_l2(*pending)
                pending = (hT, s, tok0, sz)
            emit_l2(*pending)

    nc.compile()
    return nc


def _route(cat_ids, n_cat):
    """Assign categories to cores: 2 per core, big+small paired by count."""
    counts = np.bincount(cat_ids, minlength=n_cat)
    order = np.argsort(counts, kind="stable")[::-1]  # desc by count
    seg_cats = [order[:N_CORES], order[n_cat - 1:N_CORES - 1:-1]]
    caps = []
    for j in range(2):
        mx = int(counts[seg_cats[j]].max())
        caps.append(max(CHUNK, -(-mx // CHUNK) * CHUNK))
    return seg_cats, caps, counts


_PROG_CACHE = {}


def make_in_maps(x, cat_ids, W1, b1, W2, b2):
    """Host-side routing/sharding: gather tokens by category, transpose.

    Returns (in_maps, idx_per_core, caps, dims)."""
    x = np.ascontiguousarray(np.asarray(x, dtype=np.float32))
    cat_ids = np.asarray(cat_ids)
    W1 = np.asarray(W1, dtype=np.float32)
    b1 = np.asarray(b1, dtype=np.float32)
    W2 = np.asarray(W2, dtype=np.float32)
    b2 = np.asarray(b2, dtype=np.float32)

    d_in = x.shape[1]
    n_cat, _, d_h = W1.shape
    d_out = W2.shape[2]

    seg_cats, caps, _counts = _route(cat_ids, n_cat)
    T = sum(caps)

    idx_per_core = []
    in_maps = []
    for i in range(N_CORES):
        cats = [int(seg_cats[0][i]), int(seg_cats[1][i])]
        idxs = [np.flatnonzero(cat_ids == c) for c in cats]
        idx_per_core.append(idxs)
        xT_i = np.zeros((d_in, T), dtype=np.float32)
        off = 0
        for j, (c, idx) in enumerate(zip(cats, idxs)):
            xT_i[:, off:off + len(idx)] = x[idx].T
            off += caps[j]
        b2bc = np.broadcast_to(b2[cats][:, None, :], (2, P, d_out))
        in_maps.append({
            "xT": xT_i,
            "W1": np.ascontiguousarray(W1[cats]),
            "b1": np.ascontiguousarray(b1[cats]),
            "W2": np.ascontiguousarray(W2[cats]),
            "b2bc": np.ascontiguousarray(b2bc),
        })
    return in_maps, idx_per_core, caps, (d_in, d_h, d_out)


def unshard_out(results, idx_per_core, caps, B, d_out):
    out_full = np.empty((B, d_out), dtype=np.float32)
    for i in range(N_CORES):
        o = results[i]["out"]
        off = 0
        for j, idx in enumerate(idx_per_core[i]):
            out_full[idx] = o[off:off + len(idx)]
            off += caps[j]
    return out_full


def kernel(x, cat_ids, W1, b1, W2, b2):
    in_maps, idx_per_core, caps, (d_in, d_h, d_out) = make_in_maps(
        x, cat_ids, W1, b1, W2, b2)

    key = (tuple(caps), d_in, d_h, d_out)
    if key not in _PROG_CACHE:
        _PROG_CACHE[key] = build_program(caps, d_in, d_h, d_out)
    nc = _PROG_CACHE[key]

    res = run_bass_kernel_spmd(nc, in_maps, list(range(N_CORES)))
    return unshard_out(res.results, idx_per_core, caps,
                       np.asarray(x).shape[0], d_out)
